# revision 15
# baseline (speedup 1.0000x reference)
"""Trainium2 Bass kernel for nn_AttenMlpFinal (attention-MLP pooling).

Reference (per batch row b):
    xx[m]  = concat(q[b], k[b,m])                  # [2D]
    h      = relu(xx @ W1^T)                       # [M, H]
    scores = h @ W2^T                              # [M]
    attn   = softmax(scores over m)
    out[b] = sum_m attn[m] * v[b,m]                # [D]

Strategy (pure data parallel over bsz across 8 cores; bf16 matmul inputs):
  Fold |W2_h| into W1 row h (relu scale-invariance), permute hidden units
  into three groups [act(neg) | min(neg) | max(pos)]:
    scores[b,m] = sum_pos max(K_h, -P_h) + sum_negDVE min(-K_h, P_h)
                  - sum_negACT relu(P_h + K_h)   (+ const(b) dropped:
                  q-only linear terms are constant over m and cancel in
                  softmax, so no q-replay matmuls or linear corrections).
  where P = q-side preactivation, K = k-side preactivation (|W2|-scaled).
  Engines:
    PE  (bf16, FWL): K = k.WK per (block,m); q-fold only for the ACT
        group's Q cols; nPQ = q.WQn once per block; v-sum via
        identity-stationary accumulating matmuls over attn-scaled v.
    ACT: relu+accum on the act group (Q cols, full preact in PSUM);
         psum->sbuf copies; exp.
    DVE: scalar_tensor_tensor min/max with accum on the other L cols
         (in0 = PSUM K, in1 = nPQ in SBUF); softmax combine; attn-scale
         of v in bf16 (4x packed mode).
  softmax over m=8 without max subtraction (scores are O(1)).
  k and q ship pre-transposed (kT [D,M,B], qT [D,B]) so the contraction
  dim d sits on partitions with zero on-chip transposes.
"""

import sys

sys.path.insert(0, "/opt/trn_rl_repo")

from contextlib import ExitStack

import numpy as np
import ml_dtypes

import concourse.bass as bass
import concourse.tile as tile
from concourse import bacc, mybir
from concourse.bass_utils import run_bass_kernel_spmd

F32 = mybir.dt.float32
BF16 = mybir.dt.bfloat16
ALU = mybir.AluOpType
ACTF = mybir.ActivationFunctionType

N_CORES = 8
BSZ, M, D, H = 32768, 8, 128, 512
B = BSZ // N_CORES  # rows per core

GROUP = 4  # b-blocks per v-sum matmul group (psum bank = 4*128 fp32 cols)
Q_TARGET = 512  # ACT-side hidden group size cap (clamped to #neg(W2))

BF = ml_dtypes.bfloat16


def build_nc(b_per_core: int, Q: int, r: int):
    """Q = ACT group size, r = DVE min-group size; L = H - Q total DVE cols."""
    L = H - Q
    Hp = L - r  # DVE max-group size
    nb = b_per_core // 128
    ngroups = nb // GROUP
    assert nb % GROUP == 0

    nc = bacc.Bacc("TRN2", target_bir_lowering=False, debug=False)

    kT = nc.declare_dram_parameter("kT", [D, M, b_per_core], BF16, isOutput=False)
    qT = nc.declare_dram_parameter("qT", [D, b_per_core], BF16, isOutput=False)
    v = nc.declare_dram_parameter("v", [b_per_core, M * D], BF16, isOutput=False)
    wk = nc.declare_dram_parameter("wk", [D, H], BF16, isOutput=False)
    wqa = nc.declare_dram_parameter("wqa", [D, H], BF16, isOutput=False)
    wqn = nc.declare_dram_parameter("wqn", [D, L], BF16, isOutput=False)
    ident = nc.declare_dram_parameter("ident", [128, 128], BF16, isOutput=False)
    out = nc.declare_dram_parameter("out", [b_per_core, D], F32, isOutput=True)

    with tile.TileContext(nc) as tc, ExitStack() as ctx:
        dram = ctx.enter_context(tc.tile_pool(name="dram", bufs=1, space="DRAM"))
        consts = ctx.enter_context(tc.tile_pool(name="consts", bufs=1))
        qpool = ctx.enter_context(tc.tile_pool(name="qpool", bufs=1))
        kpool = ctx.enter_context(tc.tile_pool(name="kpool", bufs=2))
        vpool = ctx.enter_context(tc.tile_pool(name="vpool", bufs=2))
        npqp = ctx.enter_context(tc.tile_pool(name="npqp", bufs=2))
        scr = ctx.enter_context(tc.tile_pool(name="scr", bufs=6))
        smax = ctx.enter_context(tc.tile_pool(name="smax", bufs=2 * GROUP + 2))
        vsc = ctx.enter_context(tc.tile_pool(name="vsc", bufs=2))
        outp = ctx.enter_context(tc.tile_pool(name="outp", bufs=2))

        ps_a = ctx.enter_context(tc.tile_pool(name="ps_a", bufs=5, space="PSUM"))
        ps_npq = ctx.enter_context(tc.tile_pool(name="ps_npq", bufs=1, space="PSUM"))
        ps_vo = ctx.enter_context(tc.tile_pool(name="ps_vo", bufs=2, space="PSUM"))

        # ---- constants ----
        wk_sb = consts.tile([D, H], BF16, tag="wk")
        nc.sync.dma_start(out=wk_sb[:], in_=wk[:])
        wqa_sb = consts.tile([D, H], BF16, tag="wqa")
        nc.sync.dma_start(out=wqa_sb[:], in_=wqa[:])
        wqn_sb = consts.tile([D, L], BF16, tag="wqn")
        nc.sync.dma_start(out=wqn_sb[:], in_=wqn[:])
        id_sb = consts.tile([128, 128], BF16, tag="ident")
        nc.sync.dma_start(out=id_sb[:], in_=ident[:])

        # Stage big inputs into internal DRAM: external (PJRT) buffers read
        # ~7x slower from SBUF-DMA than internal DRAM tensors; the bulk
        # DRAM->DRAM copy is fast.
        kT_i = dram.tile([D, M, b_per_core], BF16, name="kT_i")
        nc.sync.dma_start(out=kT_i[:], in_=kT[:])
        qT_i = dram.tile([D, b_per_core], BF16, name="qT_i")
        nc.sync.dma_start(out=qT_i[:], in_=qT[:])
        v_i = dram.tile([b_per_core, M * D], BF16, name="v_i")
        nc.sync.dma_start(out=v_i[:], in_=v[:])

        qT_sb = qpool.tile([D, b_per_core], BF16)
        nc.sync.dma_start(out=qT_sb[:], in_=qT_i[:])

        def emit_vsum(gb_prev, vscaled_prev):
            # v-sum via identity-stationary accumulating matmuls; emitted one
            # group late so these PE ops (which depend on the previous
            # group's last DVE v-scales) never head-of-line-block the PE
            # queue: by emission time their inputs are long since ready.
            vo_ps = ps_vo.tile([128, GROUP * 128], F32)
            for m in range(M):
                nc.tensor.matmul(
                    vo_ps[:],
                    id_sb[:],
                    vscaled_prev[m][:, :, :],
                    start=(m == 0),
                    stop=(m == M - 1),
                )
            out_sb = outp.tile([128, GROUP, 128], F32)
            nc.scalar.copy(out_sb[:, :, :], vo_ps[:])
            for j in range(GROUP):
                nc.sync.dma_start(
                    out=out[gb_prev + j * 128 : gb_prev + (j + 1) * 128, :],
                    in_=out_sb[:, j, :],
                )

        pending_vsum = None  # (gb, vscaled) of the previous group

        for g in range(ngroups):
            gb = g * GROUP * 128  # first b row of this group

            kT_sb = kpool.tile([D, M, GROUP * 128], BF16)
            nc.sync.dma_start(out=kT_sb[:], in_=kT_i[:, :, gb : gb + GROUP * 128])
            v_sb = vpool.tile([128, GROUP, M * D], BF16)
            for j in range(GROUP):
                nc.sync.dma_start(
                    out=v_sb[:, j, :], in_=v_i[gb + j * 128 : gb + (j + 1) * 128, :]
                )

            vscaled = [
                vsc.tile([128, GROUP, 128], BF16, tag=f"vs{m}", name=f"vs{m}")
                for m in range(M)
            ]

            for j in range(GROUP):
                qsl = qT_sb[:, gb + j * 128 : gb + (j + 1) * 128]

                # ---- q phase: nPQ for the DVE groups ----
                npq_ps = ps_npq.tile([128, L], F32)
                nc.tensor.matmul(npq_ps[:], qsl, wqn_sb[:], start=True, stop=True)
                npq_sb = npqp.tile([128, L], BF16)
                nc.vector.tensor_scalar(
                    npq_sb[:], npq_ps[:], 0.0, None, op0=ALU.bypass
                )

                sc_a = smax.tile([128, M], F32, tag="sc_a")
                sc_n = smax.tile([128, M], F32, tag="sc_n")
                sc_x = smax.tile([128, M], F32, tag="sc_x")

                # ---- per-m main work ----
                for m in range(M):
                    ksl = kT_sb[:, m, j * 128 : (j + 1) * 128]
                    a_ps = ps_a.tile([128, H], F32)
                    # q-fold (wqa zero-padded past col Q) + k preactivations
                    nc.tensor.matmul(
                        a_ps[:], qsl, wqa_sb[:], start=True, stop=False
                    )
                    nc.tensor.matmul(
                        a_ps[:], ksl, wk_sb[:], start=False, stop=True
                    )
                    t_a = scr.tile([128, Q], BF16, tag="scra")
                    nc.scalar.activation(
                        t_a[:], a_ps[:, 0:Q], ACTF.Relu,
                        accum_out=sc_a[:, m : m + 1],
                    )
                    if r > 0:
                        t_n = scr.tile([128, r], BF16, tag="scrn")
                        nc.vector.scalar_tensor_tensor(
                            out=t_n[:],
                            in0=a_ps[:, Q : Q + r],
                            scalar=0.0,
                            in1=npq_sb[:, 0:r],
                            op0=ALU.bypass,
                            op1=ALU.min,
                            accum_out=sc_n[:, m : m + 1],
                        )
                    t_x = scr.tile([128, Hp], BF16, tag="scrx")
                    nc.vector.scalar_tensor_tensor(
                        out=t_x[:],
                        in0=a_ps[:, Q + r : H],
                        scalar=0.0,
                        in1=npq_sb[:, r:L],
                        op0=ALU.bypass,
                        op1=ALU.max,
                        accum_out=sc_x[:, m : m + 1],
                    )

                # ---- scores = sc_x [+ sc_n] - sc_a ; softmax over m ----
                scores = smax.tile([128, M], F32, tag="scores")
                nc.vector.tensor_sub(scores[:], sc_x[:], sc_a[:])
                if r > 0:
                    nc.vector.tensor_add(scores[:], scores[:], sc_n[:])
                e_sb = smax.tile([128, M], F32, tag="e")
                denom = smax.tile([128, 1], F32, tag="denom")
                nc.scalar.activation(
                    e_sb[:], scores[:], ACTF.Exp, accum_out=denom[:]
                )
                rec = smax.tile([128, 1], F32, tag="recip")
                nc.vector.reciprocal(rec[:], denom[:])

                # ---- v * e_m * rec (attn fold; bf16, DVE packed mode) ----
                for m in range(M):
                    nc.vector.tensor_scalar(
                        vscaled[m][:, j, :],
                        v_sb[:, j, m * 128 : (m + 1) * 128],
                        e_sb[:, m : m + 1],
                        rec[:],
                        op0=ALU.mult,
                        op1=ALU.mult,
                    )

            if pending_vsum is not None:
                emit_vsum(*pending_vsum)
            pending_vsum = (gb, vscaled)

        emit_vsum(*pending_vsum)

    nc.compile()
    return nc


def host_prep(q_vec, k_vec, v_vec, W1, W2, b_per_core):
    """Host-side resharding + weight preprocessing (numpy only)."""
    W1 = np.asarray(W1, dtype=np.float32)
    w2 = np.asarray(W2, dtype=np.float32).reshape(-1)  # [H]

    neg = w2 < 0
    neg_idx = np.where(neg)[0]
    pos_idx = np.where(~neg)[0]
    Q = min(Q_TARGET, len(neg_idx))
    act_idx = neg_idx[:Q]
    min_idx = neg_idx[Q:]
    r = len(min_idx)
    order = np.concatenate([act_idx, min_idx, pos_idx])

    Ws = (np.abs(w2)[:, None] * W1)[order]  # [H, 2D] |W2|-folded, permuted
    Wsq, Wsk = Ws[:, :D], Ws[:, D:]

    WK = np.ascontiguousarray(Wsk.T, dtype=np.float32)  # [D, H]
    WK[:, Q : Q + r] *= -1.0  # min-group psum holds -K
    WQa = np.zeros((D, H), dtype=np.float32)  # [D, H], zero past col Q
    WQa[:, :Q] = Wsq[:Q].T
    WQn = np.ascontiguousarray(Wsq[Q:].T, dtype=np.float32)  # [D, L]
    WQn[:, r:] *= -1.0  # max-group in1 = -P (min-group keeps +P)

    wk_b = WK.astype(BF)
    wqa_b = WQa.astype(BF)
    wqn_b = WQn.astype(BF)
    ident = np.eye(128, dtype=np.float32).astype(BF)

    in_maps = []
    n_cores = len(q_vec) // b_per_core
    for c in range(n_cores):
        sl = slice(c * b_per_core, (c + 1) * b_per_core)
        k_sh = np.asarray(k_vec[sl], dtype=np.float32)
        q_sh = np.asarray(q_vec[sl], dtype=np.float32)
        v_sh = np.asarray(v_vec[sl], dtype=np.float32)
        in_maps.append(
            {
                "kT": np.ascontiguousarray(k_sh.transpose(2, 1, 0)).astype(BF),
                "qT": np.ascontiguousarray(q_sh.T).astype(BF),
                "v": np.ascontiguousarray(v_sh.reshape(b_per_core, M * D)).astype(BF),
                "wk": wk_b,
                "wqa": wqa_b,
                "wqn": wqn_b,
                "ident": ident,
            }
        )
    return in_maps, Q, r


_NC_CACHE = {}


def kernel(q_vec, k_vec, v_vec, W1, W2):
    in_maps, Q, r = host_prep(q_vec, k_vec, v_vec, W1, W2, B)
    key = (B, Q, r)
    if key not in _NC_CACHE:
        _NC_CACHE[key] = build_nc(B, Q, r)
    nc = _NC_CACHE[key]
    res = run_bass_kernel_spmd(nc, in_maps, list(range(N_CORES)))
    outs = [res.results[c]["out"] for c in range(N_CORES)]
    return np.ascontiguousarray(np.concatenate(outs, axis=0), dtype=np.float32)


if __name__ == "__main__":
    rng = np.random.default_rng(0)
    q = rng.standard_normal((BSZ, D), dtype=np.float32)
    k = rng.standard_normal((BSZ, M, D), dtype=np.float32)
    v = rng.standard_normal((BSZ, M, D), dtype=np.float32)
    W1 = (rng.standard_normal((H, 2 * D)) / np.sqrt(2 * D)).astype(np.float32)
    W2 = (rng.standard_normal((1, H)) / np.sqrt(H)).astype(np.float32)
    o = kernel(q, k, v, W1, W2)
    print(o.shape, o.dtype)


# revision 17
# speedup vs baseline: 1.0726x; 1.0726x over previous
"""Trainium2 Bass kernel for nn_AttenMlpFinal (attention-MLP pooling).

Reference (per batch row b):
    xx[m]  = concat(q[b], k[b,m])                  # [2D]
    h      = relu(xx @ W1^T)                       # [M, H]
    scores = h @ W2^T                              # [M]
    attn   = softmax(scores over m)
    out[b] = sum_m attn[m] * v[b,m]                # [D]

Strategy (pure data parallel over bsz across 8 cores; bf16 matmul inputs):
  Fold |W2_h| into W1 row h (relu scale-invariance), permute hidden units
  into three groups [act(neg) | min(neg) | max(pos)]:
    scores[b,m] = sum_pos max(K_h, -P_h) + sum_negDVE min(-K_h, P_h)
                  - sum_negACT relu(P_h + K_h)   (+ const(b) dropped:
                  q-only linear terms are constant over m and cancel in
                  softmax, so no q-replay matmuls or linear corrections).
  where P = q-side preactivation, K = k-side preactivation (|W2|-scaled).
  Engines:
    PE  (bf16, FWL): K = k.WK per (block,m); q-fold only for the ACT
        group's Q cols; nPQ = q.WQn once per block; v-sum via
        identity-stationary accumulating matmuls over attn-scaled v.
    ACT: relu+accum on the act group (Q cols, full preact in PSUM);
         psum->sbuf copies; exp.
    DVE: scalar_tensor_tensor min/max with accum on the other L cols
         (in0 = PSUM K, in1 = nPQ in SBUF); softmax combine; attn-scale
         of v in bf16 (4x packed mode).
  softmax over m=8 without max subtraction (scores are O(1)).
  k and q ship pre-transposed (kT [D,M,B], qT [D,B]) so the contraction
  dim d sits on partitions with zero on-chip transposes.
"""

import sys

sys.path.insert(0, "/opt/trn_rl_repo")

from contextlib import ExitStack

import numpy as np
import ml_dtypes

import concourse.bass as bass
import concourse.tile as tile
from concourse import bacc, mybir
from concourse.bass_utils import run_bass_kernel_spmd

F32 = mybir.dt.float32
BF16 = mybir.dt.bfloat16
ALU = mybir.AluOpType
ACTF = mybir.ActivationFunctionType

N_CORES = 8
BSZ, M, D, H = 32768, 8, 128, 512
B = BSZ // N_CORES  # rows per core

GROUP = 4  # b-blocks per v-sum matmul group (psum bank = 4*128 fp32 cols)
Q_TARGET = 512  # ACT-side hidden group size cap (clamped to #neg(W2))

BF = ml_dtypes.bfloat16


def build_nc(b_per_core: int, Q: int, r: int):
    """Q = ACT group size, r = DVE min-group size; L = H - Q total DVE cols."""
    L = H - Q
    Hp = L - r  # DVE max-group size
    nb = b_per_core // 128
    ngroups = nb // GROUP
    assert nb % GROUP == 0

    nc = bacc.Bacc("TRN2", target_bir_lowering=False, debug=False)

    kT = nc.declare_dram_parameter("kT", [D, M, b_per_core], BF16, isOutput=False)
    qT = nc.declare_dram_parameter("qT", [D, b_per_core], BF16, isOutput=False)
    v = nc.declare_dram_parameter("v", [b_per_core, M * D], BF16, isOutput=False)
    wk = nc.declare_dram_parameter("wk", [D, H], BF16, isOutput=False)
    wqa = nc.declare_dram_parameter("wqa", [D, H], BF16, isOutput=False)
    wqn = nc.declare_dram_parameter("wqn", [D, L], BF16, isOutput=False)
    ident = nc.declare_dram_parameter("ident", [128, 128], BF16, isOutput=False)
    out = nc.declare_dram_parameter("out", [b_per_core, D], F32, isOutput=True)

    with tile.TileContext(nc) as tc, ExitStack() as ctx:
        dram = ctx.enter_context(tc.tile_pool(name="dram", bufs=1, space="DRAM"))
        consts = ctx.enter_context(tc.tile_pool(name="consts", bufs=1))
        qpool = ctx.enter_context(tc.tile_pool(name="qpool", bufs=1))
        kpool = ctx.enter_context(tc.tile_pool(name="kpool", bufs=2))
        vpool = ctx.enter_context(tc.tile_pool(name="vpool", bufs=2))
        npqp = ctx.enter_context(tc.tile_pool(name="npqp", bufs=2))
        scr = ctx.enter_context(tc.tile_pool(name="scr", bufs=6))
        smax = ctx.enter_context(tc.tile_pool(name="smax", bufs=2 * GROUP + 2))
        vsc = ctx.enter_context(tc.tile_pool(name="vsc", bufs=2))
        outp = ctx.enter_context(tc.tile_pool(name="outp", bufs=2))

        ps_a = ctx.enter_context(tc.tile_pool(name="ps_a", bufs=5, space="PSUM"))
        ps_npq = ctx.enter_context(tc.tile_pool(name="ps_npq", bufs=1, space="PSUM"))
        ps_vo = ctx.enter_context(tc.tile_pool(name="ps_vo", bufs=2, space="PSUM"))

        # ---- constants ----
        wk_sb = consts.tile([D, H], BF16, tag="wk")
        nc.sync.dma_start(out=wk_sb[:], in_=wk[:])
        wqa_sb = consts.tile([D, H], BF16, tag="wqa")
        nc.sync.dma_start(out=wqa_sb[:], in_=wqa[:])
        wqn_sb = consts.tile([D, L], BF16, tag="wqn")
        nc.sync.dma_start(out=wqn_sb[:], in_=wqn[:])
        id_sb = consts.tile([128, 128], BF16, tag="ident")
        nc.sync.dma_start(out=id_sb[:], in_=ident[:])

        # Stage big inputs into internal DRAM: external (PJRT) buffers read
        # ~7x slower from SBUF-DMA than internal DRAM tensors; the bulk
        # DRAM->DRAM copy is fast.
        kT_i = dram.tile([D, M, b_per_core], BF16, name="kT_i")
        nc.sync.dma_start(out=kT_i[:], in_=kT[:])
        qT_i = dram.tile([D, b_per_core], BF16, name="qT_i")
        nc.sync.dma_start(out=qT_i[:], in_=qT[:])
        v_i = dram.tile([b_per_core, M * D], BF16, name="v_i")
        nc.sync.dma_start(out=v_i[:], in_=v[:])

        qT_sb = qpool.tile([D, b_per_core], BF16)
        nc.sync.dma_start(out=qT_sb[:], in_=qT_i[:])

        def emit_vsum(gb_prev, vscaled_prev):
            # v-sum via identity-stationary accumulating matmuls; emitted one
            # group late so these PE ops (which depend on the previous
            # group's last DVE v-scales) never head-of-line-block the PE
            # queue: by emission time their inputs are long since ready.
            vo_ps = ps_vo.tile([128, GROUP * 128], F32)
            for m in range(M):
                nc.tensor.matmul(
                    vo_ps[:],
                    id_sb[:],
                    vscaled_prev[m][:, :, :],
                    start=(m == 0),
                    stop=(m == M - 1),
                )
            out_sb = outp.tile([128, GROUP, 128], F32)
            nc.scalar.copy(out_sb[:, :, :], vo_ps[:])
            for j in range(GROUP):
                nc.sync.dma_start(
                    out=out[gb_prev + j * 128 : gb_prev + (j + 1) * 128, :],
                    in_=out_sb[:, j, :],
                )

        pending_vsum = None  # (gb, vscaled) of the previous group

        for g in range(ngroups):
            gb = g * GROUP * 128  # first b row of this group

            kT_sb = kpool.tile([D, M, GROUP * 128], BF16)
            nc.sync.dma_start(out=kT_sb[:], in_=kT_i[:, :, gb : gb + GROUP * 128])
            v_sb = vpool.tile([128, GROUP, M * D], BF16)
            for j in range(GROUP):
                nc.sync.dma_start(
                    out=v_sb[:, j, :], in_=v_i[gb + j * 128 : gb + (j + 1) * 128, :]
                )

            vscaled = [
                vsc.tile([128, GROUP, 128], BF16, tag=f"vs{m}", name=f"vs{m}")
                for m in range(M)
            ]

            for j in range(GROUP):
                qsl = qT_sb[:, gb + j * 128 : gb + (j + 1) * 128]

                # ---- q phase: nPQ for the DVE groups ----
                npq_ps = ps_npq.tile([128, L], F32)
                nc.tensor.matmul(npq_ps[:], qsl, wqn_sb[:], start=True, stop=True)
                npq_sb = npqp.tile([128, L], BF16)
                nc.vector.tensor_scalar(
                    npq_sb[:], npq_ps[:], 0.0, None, op0=ALU.bypass
                )

                sc_a = smax.tile([128, M], F32, tag="sc_a")
                sc_n = smax.tile([128, M], F32, tag="sc_n")
                sc_x = smax.tile([128, M], F32, tag="sc_x")

                # ---- per-m main work ----
                for m in range(M):
                    ksl = kT_sb[:, m, j * 128 : (j + 1) * 128]
                    a_ps = ps_a.tile([128, H], F32)
                    # q-fold streams only its Q useful cols; the full-width
                    # k matmul runs start=False: PSUM has_written bits make
                    # it accumulate onto cols :Q and fresh-write cols Q:.
                    nc.tensor.matmul(
                        a_ps[:, 0:Q], qsl, wqa_sb[:, 0:Q],
                        start=True, stop=False, skip_group_check=True,
                    )
                    nc.tensor.matmul(
                        a_ps[:], ksl, wk_sb[:],
                        start=False, stop=True, skip_group_check=True,
                    )
                    t_a = scr.tile([128, Q], BF16, tag="scra")
                    nc.scalar.activation(
                        t_a[:], a_ps[:, 0:Q], ACTF.Relu,
                        accum_out=sc_a[:, m : m + 1],
                    )
                    if r > 0:
                        t_n = scr.tile([128, r], BF16, tag="scrn")
                        nc.vector.scalar_tensor_tensor(
                            out=t_n[:],
                            in0=a_ps[:, Q : Q + r],
                            scalar=0.0,
                            in1=npq_sb[:, 0:r],
                            op0=ALU.bypass,
                            op1=ALU.min,
                            accum_out=sc_n[:, m : m + 1],
                        )
                    t_x = scr.tile([128, Hp], BF16, tag="scrx")
                    nc.vector.scalar_tensor_tensor(
                        out=t_x[:],
                        in0=a_ps[:, Q + r : H],
                        scalar=0.0,
                        in1=npq_sb[:, r:L],
                        op0=ALU.bypass,
                        op1=ALU.max,
                        accum_out=sc_x[:, m : m + 1],
                    )

                # ---- scores = sc_x [+ sc_n] - sc_a ; softmax over m ----
                scores = smax.tile([128, M], F32, tag="scores")
                nc.vector.tensor_sub(scores[:], sc_x[:], sc_a[:])
                if r > 0:
                    nc.vector.tensor_add(scores[:], scores[:], sc_n[:])
                e_sb = smax.tile([128, M], F32, tag="e")
                nc.scalar.activation(e_sb[:], scores[:], ACTF.Exp)
                denom = smax.tile([128, 1], F32, tag="denom")
                nc.vector.tensor_reduce(
                    denom[:], e_sb[:], mybir.AxisListType.X, ALU.add
                )
                rec = smax.tile([128, 1], F32, tag="recip")
                nc.vector.reciprocal(rec[:], denom[:])
                attn = smax.tile([128, M], F32, tag="attn")
                nc.vector.tensor_scalar_mul(attn[:], e_sb[:], rec[:])

                # ---- scale v by attn_m (bf16, DVE packed mode) ----
                for m in range(M):
                    nc.vector.tensor_scalar_mul(
                        vscaled[m][:, j, :],
                        v_sb[:, j, m * 128 : (m + 1) * 128],
                        attn[:, m : m + 1],
                    )

            if pending_vsum is not None:
                emit_vsum(*pending_vsum)
            pending_vsum = (gb, vscaled)

        emit_vsum(*pending_vsum)

    nc.compile()
    return nc


def host_prep(q_vec, k_vec, v_vec, W1, W2, b_per_core):
    """Host-side resharding + weight preprocessing (numpy only)."""
    W1 = np.asarray(W1, dtype=np.float32)
    w2 = np.asarray(W2, dtype=np.float32).reshape(-1)  # [H]

    neg = w2 < 0
    neg_idx = np.where(neg)[0]
    pos_idx = np.where(~neg)[0]
    Q = min(Q_TARGET, len(neg_idx))
    act_idx = neg_idx[:Q]
    min_idx = neg_idx[Q:]
    r = len(min_idx)
    order = np.concatenate([act_idx, min_idx, pos_idx])

    Ws = (np.abs(w2)[:, None] * W1)[order]  # [H, 2D] |W2|-folded, permuted
    Wsq, Wsk = Ws[:, :D], Ws[:, D:]

    WK = np.ascontiguousarray(Wsk.T, dtype=np.float32)  # [D, H]
    WK[:, Q : Q + r] *= -1.0  # min-group psum holds -K
    WQa = np.zeros((D, H), dtype=np.float32)  # [D, H], zero past col Q
    WQa[:, :Q] = Wsq[:Q].T
    WQn = np.ascontiguousarray(Wsq[Q:].T, dtype=np.float32)  # [D, L]
    WQn[:, r:] *= -1.0  # max-group in1 = -P (min-group keeps +P)

    wk_b = WK.astype(BF)
    wqa_b = WQa.astype(BF)
    wqn_b = WQn.astype(BF)
    ident = np.eye(128, dtype=np.float32).astype(BF)

    in_maps = []
    n_cores = len(q_vec) // b_per_core
    for c in range(n_cores):
        sl = slice(c * b_per_core, (c + 1) * b_per_core)
        k_sh = np.asarray(k_vec[sl], dtype=np.float32)
        q_sh = np.asarray(q_vec[sl], dtype=np.float32)
        v_sh = np.asarray(v_vec[sl], dtype=np.float32)
        in_maps.append(
            {
                "kT": np.ascontiguousarray(k_sh.transpose(2, 1, 0)).astype(BF),
                "qT": np.ascontiguousarray(q_sh.T).astype(BF),
                "v": np.ascontiguousarray(v_sh.reshape(b_per_core, M * D)).astype(BF),
                "wk": wk_b,
                "wqa": wqa_b,
                "wqn": wqn_b,
                "ident": ident,
            }
        )
    return in_maps, Q, r


_NC_CACHE = {}


def kernel(q_vec, k_vec, v_vec, W1, W2):
    in_maps, Q, r = host_prep(q_vec, k_vec, v_vec, W1, W2, B)
    key = (B, Q, r)
    if key not in _NC_CACHE:
        _NC_CACHE[key] = build_nc(B, Q, r)
    nc = _NC_CACHE[key]
    res = run_bass_kernel_spmd(nc, in_maps, list(range(N_CORES)))
    outs = [res.results[c]["out"] for c in range(N_CORES)]
    return np.ascontiguousarray(np.concatenate(outs, axis=0), dtype=np.float32)


if __name__ == "__main__":
    rng = np.random.default_rng(0)
    q = rng.standard_normal((BSZ, D), dtype=np.float32)
    k = rng.standard_normal((BSZ, M, D), dtype=np.float32)
    v = rng.standard_normal((BSZ, M, D), dtype=np.float32)
    W1 = (rng.standard_normal((H, 2 * D)) / np.sqrt(2 * D)).astype(np.float32)
    W2 = (rng.standard_normal((1, H)) / np.sqrt(H)).astype(np.float32)
    o = kernel(q, k, v, W1, W2)
    print(o.shape, o.dtype)
